# revision 35
# baseline (speedup 1.0000x reference)
"""Trainium2 Bass kernel for nn_CombinatorialClassifier (segment_reduce).

Strategy (8 NeuronCores, tensor-parallel over the num_partitionings axis):
  Core i owns partitionings {2i, 2i+1} (a [2000, 2048] slice of W).
  On device:
    1. logits = x @ Wshard.T + b  (PE, fp16)
    2. per-partitioning softmax -> probs [64, 2048] fp16 (padded 1024/stream)
    3. probs transposed via PE -> probsT [128, 16, 64] (row k at partition
       k%128, chunk k//128; stream s rows at 1024s+k)
    4. "gather" as staircase one-hot matmuls: host sorts classes of each
       partitioning by partition idx k; each 32-row block of probsT is
       multiplied with a one-hot fp8 block S [32, NP] whose column j selects
       row k of the block -> PSUM [64, NP] = probs gathered in sorted order.
       Streams a/b go to PSUM partitions 0-63 / 64-127 (PE column tiling),
       evacuated to fp16 and DMA'd to DRAM in padded sorted order.
  Host: un-permute the 16 padded sorted streams, sum, normalize, log.

The Q7 ap_gather of the previous version (27 ns/idx, 1.43 ms total) is
replaced by PE matmuls at ~0.5 ns/gathered element.
"""

import os
from contextlib import ExitStack

import numpy as np

import concourse.bacc as bacc
import concourse.mybir as mybir
import concourse.tile as tile
from concourse import bass_utils

B, P, K, C, D = 64, 16, 1000, 50000, 2048
ESP = 1e-20
NCORES = 8
PPC = P // NCORES        # partitionings per core (2)
NLOC = PPC * K           # local logits width (2000)
NT = 500                 # logits matmul N-tile (PSUM bank: 500 fp32 <= 512)
NNT = NLOC // NT         # 4 N-tiles
DCH = D // 128           # 16 contraction chunks of 128
KP = 1024                # padded rows per stream (k space)
NBLK = KP // 32          # 32-row blocks per stream
NP = 1792                # padded sorted-class columns per block
NPAD = NBLK * NP         # padded sorted stream length (57344)

_F32 = mybir.dt.float32
_F16 = mybir.dt.float16
_F8 = mybir.dt.float8e4
_F8NP = mybir.dt.np(_F8)

_CACHE = {}
LAST_RESULTS = None


def _build_nc():
    nc = bacc.Bacc(
        "TRN2",
        target_bir_lowering=False,
        debug=False,
        enable_asserts=False,
        num_devices=NCORES,
    )
    xtp_d = nc.dram_tensor("xtp", [128, DCH, B], _F16, kind="ExternalInput")
    w8_d = nc.dram_tensor("w8", [D, NLOC], _F8, kind="ExternalInput")
    bias_d = nc.dram_tensor("bias", [1, NLOC], _F16, kind="ExternalInput")
    s_d = nc.dram_tensor("sel", [128, 16, NP], _F8, kind="ExternalInput")
    id_d = nc.dram_tensor("id64", [B, B], _F16, kind="ExternalInput")
    out_d = nc.dram_tensor("pout", [PPC, B, NPAD], _F8, kind="ExternalOutput")
    probs_d = nc.dram_tensor("probs16", [B, 2 * KP], _F16, kind="ExternalOutput")

    with tile.TileContext(nc) as tc, ExitStack() as ctx:
        const = ctx.enter_context(tc.tile_pool(name="const", bufs=1))
        wpool = ctx.enter_context(tc.tile_pool(name="w", bufs=3))
        spool = ctx.enter_context(tc.tile_pool(name="stats", bufs=1))
        opool = ctx.enter_context(tc.tile_pool(name="o", bufs=3))

        xt = const.tile([128, DCH, B], _F16)
        nc.sync.dma_start(xt[:], xtp_d.ap())
        ones = const.tile([1, B], _F16)
        bias = const.tile([1, NLOC], _F16)
        id64 = const.tile([B, B], _F16)
        sel = const.tile([128, 16, NP], _F8)

        probs16 = const.tile([B, 2 * KP], _F16)
        probsT = const.tile([128, 16, B], _F16)
        recP = const.tile([128, 1], _F32)

        # ---- phase 1: logits = x @ Wshard.T + b, softmax -> probs16 ----
        with tc.tile_pool(name="psum1", bufs=1, space="PSUM") as psum1:
            ps = [
                psum1.tile([B, NT], _F32, tag=f"ps{n}", name=f"ps{n}")
                for n in range(NNT)
            ]
            # W in 8 group-DMAs of 2 chunks each, alternating dispatch
            # queues (Sync/DVE) so the descriptor rings don't serialize
            for g in range(8):
                wt = wpool.tile([128, 2, NLOC], _F8, tag="wt", name=f"wt{g}")
                eng = nc.sync if g % 2 == 0 else nc.scalar
                eng.dma_start(
                    wt[:],
                    w8_d[256 * g : 256 * (g + 1), :].rearrange(
                        "(c p) n -> p c n", p=128
                    ),
                )
                for jj in range(2):
                    j = 2 * g + jj
                    for n in range(NNT):
                        nc.tensor.matmul(
                            ps[n][:],
                            xt[:, j, :],
                            wt[:, jj, NT * n : NT * (n + 1)],
                            start=(j == 0),
                            stop=False,
                        )
            # phase-2 / small inputs: off the critical W path, on the
            # gpsimd and scalar queues (idle during phase 1)
            nc.sync.dma_start(sel[:], s_d.ap())
            nc.gpsimd.dma_start(bias[:], bias_d.ap())
            nc.gpsimd.dma_start(id64[:], id_d.ap())
            nc.vector.memset(ones[:], 1.0)
            nc.vector.memset(probs16[:], 0.0)
            for n in range(NNT):
                nc.tensor.matmul(
                    ps[n][:],
                    ones[:],
                    bias[:, NT * n : NT * (n + 1)],
                    start=False,
                    stop=True,
                )

            # logits are bounded (|l| < ~7 for this problem scale), so exp
            # without max-subtraction is safe and exp fits fp16; the
            # softmax 1/Z normalization is folded into the evac scale
            sacc = spool.tile([B, NNT], _F32)
            # bank n holds logits cols [500n, 500n+500) of the k-contiguous
            # [0, 2000) space; padded target col = 1024h + k
            goff = [0, 500, KP, KP + 500]
            for n in range(NNT):
                nc.scalar.activation(
                    probs16[0:B, goff[n] : goff[n] + NT],
                    ps[n][:],
                    mybir.ActivationFunctionType.Exp,
                    accum_out=sacc[:, n : n + 1],
                )
        rec = spool.tile([B, PPC], _F32)
        for h in range(PPC):
            nc.vector.tensor_tensor(
                rec[:, h : h + 1],
                sacc[:, 2 * h : 2 * h + 1],
                sacc[:, 2 * h + 1 : 2 * h + 2],
                op=mybir.AluOpType.add,
            )
        nc.vector.reciprocal(rec[:], rec[:])
        # recP[64s + b] = 256 / Z[b, s]  (per-partition evac scale; the
        # pout partition layout is (stream, batch))
        for h in range(PPC):
            nc.sync.dma_start(recP[B * h : B * h + B, 0:1], rec[:, h : h + 1])
        nc.vector.tensor_scalar_mul(recP[:], recP[:], 256.0)
        nc.sync.dma_start(probs_d.ap(), probs16[:, :])

        # ---- phase 1.5: transpose probs16 -> probsT [128, 16, 64] ----
        # stream-a and stream-b chunks interleaved so phase 2's first blocks
        # unblock as early as possible; pool closes to free its PSUM banks
        with tc.tile_pool(name="psumt", bufs=2, space="PSUM") as psumt:
            for cc in range(16):
                c = (cc // 2) + 8 * (cc % 2)
                tp = psumt.tile([128, B], _F16, tag="tp")
                nc.tensor.transpose(
                    tp[:, :], probs16[:, 128 * c : 128 * (c + 1)], id64[:, :]
                )
                nc.scalar.activation(
                    probsT[:, c, :], tp[:, :],
                    mybir.ActivationFunctionType.Copy,
                )

        psum2 = ctx.enter_context(
            tc.tile_pool(name="psum2", bufs=2, space="PSUM")
        )

        # ---- phase 2: staircase one-hot gather matmuls + evac + store ----
        # four t-blocks interleaved -> 8 PE sub-tiles (4 rows x 2 cols) active
        for tt in range(0, NBLK, 4):
            otp = opool.tile([128, 4, NP], _F8, tag="otp", name=f"otp{tt}")
            for j in range(4):
                w = 512 if j < 3 else NP - 3 * 512
                for u in range(4):
                    t = tt + u
                    prow = 32 * (t % 4)
                    chunk = t // 4
                    pst = psum2.tile(
                        [128, 512], _F32, tag=f"ps{u}", name=f"ps{u}_{tt}_{j}"
                    )
                    for strm in range(PPC):
                        nc.tensor.matmul(
                            pst[B * strm : B * strm + B, 0:w],
                            probsT[prow : prow + 32, 8 * strm + chunk, :],
                            sel[
                                prow : prow + 32,
                                8 * strm + chunk,
                                512 * j : 512 * j + w,
                            ],
                            start=True,
                            stop=True,
                            tile_position=(prow, B * strm),
                        )
                    # evac scale = 256/Z: softmax normalization + fp8 range
                    if (t * 4 + j) % 2 == 0:
                        nc.vector.tensor_scalar_mul(
                            otp[:, u, 512 * j : 512 * j + w],
                            pst[:, 0:w],
                            recP[:, 0:1],
                        )
                    else:
                        nc.scalar.activation(
                            otp[:, u, 512 * j : 512 * j + w],
                            pst[:, 0:w],
                            mybir.ActivationFunctionType.Copy,
                            scale=recP[:, 0:1],
                        )
            nc.sync.dma_start(
                out_d.ap()
                .rearrange("s b np -> (s b) np")[:, NP * tt : NP * (tt + 4)],
                otp[:, :, :],
            )

    nc.compile()
    return nc


def _host_inputs(x, W, b, part):
    """Per-core inputs + per-(core,stream) position maps for host unpermute."""
    xtp = np.ascontiguousarray(
        x.T.astype(np.float16).reshape(DCH, 128, B).transpose(1, 0, 2)
    )
    id64 = np.eye(B, dtype=np.float16)
    part = np.asarray(part).astype(np.int64, copy=False)
    in_maps = []
    pos_maps = []       # [core][strm] -> int32 [C] padded position or -1
    overflows = []      # [core][strm] -> list of (class, k) fallen out of NP
    for i in range(NCORES):
        r0 = NLOC * i
        w8 = W[r0 : r0 + NLOC].T.astype(_F8NP)
        bias = b[r0 : r0 + NLOC].astype(np.float16)[None, :]

        sel = np.zeros((128, 16, NP), _F8NP)
        pm_core = []
        ov_core = []
        for s in range(PPC):
            kloc = part[PPC * i + s] - (PPC * i + s) * K  # [C] in [0, K)
            order = np.argsort(kloc, kind="stable")
            k_sorted = kloc[order]
            blk = k_sorted >> 5                            # 32-row block id
            # rank within block
            starts = np.searchsorted(blk, np.arange(NBLK))
            rank = np.arange(C, dtype=np.int64) - starts[blk]
            ok = rank < NP
            prow = 32 * (blk % 4) + (k_sorted & 31)
            dim1 = 8 * s + (blk >> 2)
            sel[prow[ok], dim1[ok], rank[ok]] = 1.0
            pos = np.where(ok, blk * NP + rank, -1).astype(np.int64)
            pm = np.empty(C, np.int64)
            pm[order] = pos
            pm_core.append(pm)
            if not ok.all():
                bad = order[~ok]
                ov_core.append([(int(c), int(kloc[c])) for c in bad])
            else:
                ov_core.append([])
        pos_maps.append(pm_core)
        overflows.append(ov_core)
        in_maps.append(
            {"xtp": xtp, "w8": w8, "bias": bias, "sel": sel, "id64": id64}
        )
    return in_maps, pos_maps, overflows


def kernel(**inputs):
    global LAST_RESULTS
    x = np.asarray(inputs["input"], dtype=np.float32)
    W = np.asarray(inputs["W"], dtype=np.float32)
    b = np.asarray(inputs["b"], dtype=np.float32)
    part = np.asarray(inputs["partitionings"])
    assert x.shape == (B, D) and W.shape == (P * K, D)

    if "nc" not in _CACHE:
        _CACHE["nc"] = _build_nc()
    nc = _CACHE["nc"]

    in_maps, pos_maps, overflows = _host_inputs(x, W, b, part)
    trace = bool(int(os.environ.get("BASSK_TRACE", "0")))
    res = bass_utils.run_bass_kernel_spmd(
        nc,
        in_maps,
        core_ids=list(range(NCORES)),
        trace=trace,
        tmpdir=os.environ.get("BASSK_TRACE_DIR") or None,
    )
    LAST_RESULTS = res

    acc = np.zeros((B, C), np.float32)
    for i in range(NCORES):
        pout = res.results[i]["pout"]          # [PPC, B, NPAD] fp8 (x256)
        for s in range(PPC):
            acc += pout[s][:, pos_maps[i][s]].astype(np.float32) * (1 / 256.0)
            if overflows[i][s]:
                # probs16 holds unnormalized exp values; normalize here
                pr = res.results[i]["probs16"].astype(np.float32)
                z = pr[:, KP * s : KP * s + K].sum(axis=1)
                for c, k in overflows[i][s]:
                    acc[:, c] += pr[:, KP * s + k] / z
    tot = acc.sum(axis=1, keepdims=True)
    return np.log(acc / tot + ESP).astype(np.float32)


# revision 39
# speedup vs baseline: 1.2401x; 1.2401x over previous
"""Trainium2 Bass kernel for nn_CombinatorialClassifier (segment_reduce).

Strategy (8 NeuronCores, tensor-parallel over the num_partitionings axis):
  Core i owns partitionings {2i, 2i+1} (a [2000, 2048] slice of W).
  On device:
    1. logits = x @ Wshard.T + b  (PE, fp16)
    2. per-partitioning softmax -> probs [64, 2048] fp16 (padded 1024/stream)
    3. probs transposed via PE -> probsT [128, 16, 64] (row k at partition
       k%128, chunk k//128; stream s rows at 1024s+k)
    4. "gather" as staircase one-hot matmuls: host sorts classes of each
       partitioning by partition idx k; each 32-row block of probsT is
       multiplied with a one-hot fp8 block S [32, NP] whose column j selects
       row k of the block -> PSUM [64, NP] = probs gathered in sorted order.
       Streams a/b go to PSUM partitions 0-63 / 64-127 (PE column tiling),
       evacuated to fp16 and DMA'd to DRAM in padded sorted order.
  Host: un-permute the 16 padded sorted streams, sum, normalize, log.

The Q7 ap_gather of the previous version (27 ns/idx, 1.43 ms total) is
replaced by PE matmuls at ~0.5 ns/gathered element.
"""

import os
from contextlib import ExitStack

import numpy as np

import concourse.bacc as bacc
import concourse.mybir as mybir
import concourse.tile as tile
from concourse import bass_utils

B, P, K, C, D = 64, 16, 1000, 50000, 2048
ESP = 1e-20
NCORES = 8
PPC = P // NCORES        # partitionings per core (2)
NLOC = PPC * K           # local logits width (2000)
NT = 500                 # logits matmul N-tile (PSUM bank: 500 fp32 <= 512)
NNT = NLOC // NT         # 4 N-tiles
DCH = D // 128           # 16 contraction chunks of 128
KP = 1024                # padded rows per stream (k space)
NBLK = KP // 32          # 32-row blocks per stream
NP = 1792                # padded sorted-class columns per block
NPAD = NBLK * NP         # padded sorted stream length (57344)

_F32 = mybir.dt.float32
_F16 = mybir.dt.float16
_F8 = mybir.dt.float8e4
_F8NP = mybir.dt.np(_F8)

_CACHE = {}
LAST_RESULTS = None


def _build_nc():
    nc = bacc.Bacc(
        "TRN2",
        target_bir_lowering=False,
        debug=False,
        enable_asserts=False,
        num_devices=NCORES,
    )
    xtp_d = nc.dram_tensor("xtp", [128, DCH, B], _F16, kind="ExternalInput")
    w8_d = nc.dram_tensor("w8", [D, NLOC], _F8, kind="ExternalInput")
    bias_d = nc.dram_tensor("bias", [1, NLOC], _F16, kind="ExternalInput")
    s_d = nc.dram_tensor("sel", [128, 16, NP], _F8, kind="ExternalInput")
    id_d = nc.dram_tensor("id64", [B, B], _F16, kind="ExternalInput")
    out_d = nc.dram_tensor("pout", [PPC, B, NPAD], _F8, kind="ExternalOutput")
    probs_d = nc.dram_tensor("probs16", [B, 2 * KP], _F16, kind="ExternalOutput")

    with tile.TileContext(nc) as tc, ExitStack() as ctx:
        const = ctx.enter_context(tc.tile_pool(name="const", bufs=1))
        wpool = ctx.enter_context(tc.tile_pool(name="w", bufs=3))
        spool = ctx.enter_context(tc.tile_pool(name="stats", bufs=1))
        opool = ctx.enter_context(tc.tile_pool(name="o", bufs=3))

        xt = const.tile([128, DCH, B], _F16)
        nc.sync.dma_start(xt[:], xtp_d.ap())
        ones = const.tile([1, B], _F16)
        bias = const.tile([1, NLOC], _F16)
        id64 = const.tile([B, B], _F16)
        sel = const.tile([128, 16, NP], _F8)

        probs16 = const.tile([B, 2 * KP], _F16)
        probsT = const.tile([128, 16, B], _F16)

        # ---- phase 1: logits = x @ Wshard.T + b, softmax -> probs16 ----
        with tc.tile_pool(name="psum1", bufs=1, space="PSUM") as psum1:
            ps = [
                psum1.tile([B, NT], _F32, tag=f"ps{n}", name=f"ps{n}")
                for n in range(NNT)
            ]
            # W in 8 group-DMAs of 2 chunks each, alternating dispatch
            # queues (Sync/DVE) so the descriptor rings don't serialize
            for g in range(8):
                wt = wpool.tile([128, 2, NLOC], _F8, tag="wt", name=f"wt{g}")
                eng = nc.sync if g % 2 == 0 else nc.scalar
                eng.dma_start(
                    wt[:],
                    w8_d[256 * g : 256 * (g + 1), :].rearrange(
                        "(c p) n -> p c n", p=128
                    ),
                )
                for jj in range(2):
                    j = 2 * g + jj
                    for n in range(NNT):
                        nc.tensor.matmul(
                            ps[n][:],
                            xt[:, j, :],
                            wt[:, jj, NT * n : NT * (n + 1)],
                            start=(j == 0),
                            stop=False,
                        )
            # phase-2 / small inputs: off the critical W path, on the
            # gpsimd and scalar queues (idle during phase 1)
            nc.sync.dma_start(sel[:], s_d.ap())
            nc.gpsimd.dma_start(bias[:], bias_d.ap())
            nc.gpsimd.dma_start(id64[:], id_d.ap())
            nc.vector.memset(ones[:], 1.0)
            nc.vector.memset(probs16[:], 0.0)
            for n in range(NNT):
                nc.tensor.matmul(
                    ps[n][:],
                    ones[:],
                    bias[:, NT * n : NT * (n + 1)],
                    start=False,
                    stop=True,
                )

            # logits are bounded (|l| < ~7 for this problem scale), so exp
            # without max-subtraction is safe and exp fits fp16; the
            # softmax 1/Z normalization is folded into the evac scale
            sacc = spool.tile([B, NNT], _F32)
            # bank n holds logits cols [500n, 500n+500) of the k-contiguous
            # [0, 2000) space; padded target col = 1024h + k
            goff = [0, 500, KP, KP + 500]
            for n in range(NNT):
                nc.scalar.activation(
                    probs16[0:B, goff[n] : goff[n] + NT],
                    ps[n][:],
                    mybir.ActivationFunctionType.Exp,
                    accum_out=sacc[:, n : n + 1],
                )
        rec = spool.tile([B, PPC], _F32)
        for h in range(PPC):
            nc.vector.tensor_tensor(
                rec[:, h : h + 1],
                sacc[:, 2 * h : 2 * h + 1],
                sacc[:, 2 * h + 1 : 2 * h + 2],
                op=mybir.AluOpType.add,
            )
        nc.vector.reciprocal(rec[:], rec[:])
        nc.vector.tensor_scalar_mul(rec[:], rec[:], 256.0)
        # normalize in place: probs16 = 256 * softmax (fp8-friendly range);
        # evacs are then plain copies
        for h in range(PPC):
            nc.vector.tensor_scalar_mul(
                probs16[0:B, KP * h : KP * h + K],
                probs16[0:B, KP * h : KP * h + K],
                rec[:, h : h + 1],
            )
        nc.sync.dma_start(probs_d.ap(), probs16[:, :])

        # ---- phase 1.5: transpose probs16 -> probsT [128, 16, 64] ----
        # stream-a and stream-b chunks interleaved so phase 2's first blocks
        # unblock as early as possible; pool closes to free its PSUM banks
        with tc.tile_pool(name="psumt", bufs=2, space="PSUM") as psumt:
            for cc in range(16):
                c = (cc // 2) + 8 * (cc % 2)
                tp = psumt.tile([128, B], _F16, tag="tp")
                nc.tensor.transpose(
                    tp[:, :], probs16[:, 128 * c : 128 * (c + 1)], id64[:, :]
                )
                nc.scalar.activation(
                    probsT[:, c, :], tp[:, :],
                    mybir.ActivationFunctionType.Copy,
                )

        psum2 = ctx.enter_context(
            tc.tile_pool(name="psum2", bufs=2, space="PSUM")
        )

        # ---- phase 2: staircase one-hot gather matmuls + evac + store ----
        # four t-blocks interleaved -> 8 PE sub-tiles (4 rows x 2 cols) active
        for tt in range(0, NBLK, 4):
            otp = opool.tile([128, 4, NP], _F8, tag="otp", name=f"otp{tt}")
            for j in range(4):
                w = 512 if j < 3 else NP - 3 * 512
                for u in range(4):
                    t = tt + u
                    prow = 32 * (t % 4)
                    chunk = t // 4
                    pst = psum2.tile(
                        [128, 512], _F32, tag=f"ps{u}", name=f"ps{u}_{tt}_{j}"
                    )
                    for strm in range(PPC):
                        nc.tensor.matmul(
                            pst[B * strm : B * strm + B, 0:w],
                            probsT[prow : prow + 32, 8 * strm + chunk, :],
                            sel[
                                prow : prow + 32,
                                8 * strm + chunk,
                                512 * j : 512 * j + w,
                            ],
                            start=True,
                            stop=True,
                            tile_position=(prow, B * strm),
                        )
                    # plain-copy evac (values already 256*prob)
                    if (t * 4 + j) % 2 == 0:
                        nc.vector.tensor_scalar_mul(
                            otp[:, u, 512 * j : 512 * j + w], pst[:, 0:w], 1.0
                        )
                    else:
                        nc.scalar.activation(
                            otp[:, u, 512 * j : 512 * j + w],
                            pst[:, 0:w],
                            mybir.ActivationFunctionType.Copy,
                        )
            nc.sync.dma_start(
                out_d.ap()
                .rearrange("s b np -> (s b) np")[:, NP * tt : NP * (tt + 4)],
                otp[:, :, :],
            )

    nc.compile()
    return nc


def _host_inputs(x, W, b, part):
    """Per-core inputs + per-(core,stream) position maps for host unpermute."""
    xtp = np.ascontiguousarray(
        x.T.astype(np.float16).reshape(DCH, 128, B).transpose(1, 0, 2)
    )
    id64 = np.eye(B, dtype=np.float16)
    part = np.asarray(part).astype(np.int64, copy=False)
    in_maps = []
    pos_maps = []       # [core][strm] -> int32 [C] padded position or -1
    overflows = []      # [core][strm] -> list of (class, k) fallen out of NP
    for i in range(NCORES):
        r0 = NLOC * i
        w8 = W[r0 : r0 + NLOC].T.astype(_F8NP)
        bias = b[r0 : r0 + NLOC].astype(np.float16)[None, :]

        sel = np.zeros((128, 16, NP), _F8NP)
        pm_core = []
        ov_core = []
        for s in range(PPC):
            kloc = part[PPC * i + s] - (PPC * i + s) * K  # [C] in [0, K)
            order = np.argsort(kloc, kind="stable")
            k_sorted = kloc[order]
            blk = k_sorted >> 5                            # 32-row block id
            # rank within block
            starts = np.searchsorted(blk, np.arange(NBLK))
            rank = np.arange(C, dtype=np.int64) - starts[blk]
            ok = rank < NP
            prow = 32 * (blk % 4) + (k_sorted & 31)
            dim1 = 8 * s + (blk >> 2)
            sel[prow[ok], dim1[ok], rank[ok]] = 1.0
            pos = np.where(ok, blk * NP + rank, -1).astype(np.int64)
            pm = np.empty(C, np.int64)
            pm[order] = pos
            pm_core.append(pm)
            if not ok.all():
                bad = order[~ok]
                ov_core.append([(int(c), int(kloc[c])) for c in bad])
            else:
                ov_core.append([])
        pos_maps.append(pm_core)
        overflows.append(ov_core)
        in_maps.append(
            {"xtp": xtp, "w8": w8, "bias": bias, "sel": sel, "id64": id64}
        )
    return in_maps, pos_maps, overflows


def kernel(**inputs):
    global LAST_RESULTS
    x = np.asarray(inputs["input"], dtype=np.float32)
    W = np.asarray(inputs["W"], dtype=np.float32)
    b = np.asarray(inputs["b"], dtype=np.float32)
    part = np.asarray(inputs["partitionings"])
    assert x.shape == (B, D) and W.shape == (P * K, D)

    if "nc" not in _CACHE:
        _CACHE["nc"] = _build_nc()
    nc = _CACHE["nc"]

    in_maps, pos_maps, overflows = _host_inputs(x, W, b, part)
    trace = bool(int(os.environ.get("BASSK_TRACE", "0")))
    res = bass_utils.run_bass_kernel_spmd(
        nc,
        in_maps,
        core_ids=list(range(NCORES)),
        trace=trace,
        tmpdir=os.environ.get("BASSK_TRACE_DIR") or None,
    )
    LAST_RESULTS = res

    acc = np.zeros((B, C), np.float32)
    for i in range(NCORES):
        pout = res.results[i]["pout"]          # [PPC, B, NPAD] fp8 (x256)
        for s in range(PPC):
            acc += pout[s][:, pos_maps[i][s]].astype(np.float32) * (1 / 256.0)
            if overflows[i][s]:
                # probs16 holds 256 * softmax probabilities
                pr = res.results[i]["probs16"].astype(np.float32)
                for c, k in overflows[i][s]:
                    acc[:, c] += pr[:, KP * s + k] * (1 / 256.0)
    tot = acc.sum(axis=1, keepdims=True)
    return np.log(acc / tot + ESP).astype(np.float32)
